# revision 26
# baseline (speedup 1.0000x reference)
"""LocalPatchAttention Trainium2 kernel (v3: no-gather stats + half pipeline).

Data-parallel over batch B=8 across 8 NeuronCores (one image per core).
q and out live in DRAM as [128, 32768] with partitions = (channel,
row-parity): p<64 = channel p of even rows, p>=64 = odd rows; host packs
q to bf16 and unpacks f32 out.

LayerNorm is folded into the logits matmul:
  logits[v,px] = A^T(q*rr) - sA[v]*(mu[px]*rr[px]),  sA = column sums of A.

Stats are computed in two halves of 32 blocks (block = 2 row-pairs = 4
image rows = one [128,512] q tile). Per half, selector-lhsT matmuls
accumulate per-row mean into partitions 0:64 and E[q^2] into 64:128 of a
single shared PSUM bank; a ~10-op bulk DVE pipeline computes rr/mu*rr for
all 64 rows at once into rrmrX [64,1024] bf16 (cols 0:512 rr, 512:1024
mu*rr). Phase B reads each block's stat rows DIRECTLY from rrmrX with
K=32 row-selecting lhsT matmuls (rhs partition base 0 or 32; selector
tiles are duplicated at partition offset 32 to satisfy lhsT.base ==
rhs.base) -- no gather DMAs at all. Halves are pipelined: half 1's stats
matmuls fill PE gaps during half 0's phase B.

Per block B: one K=32 bcast matmul -> rr2 [128,512]; one DVE multiply
qs = q*rr2 (bf16); lg_{e,o} = A^T qs accumulated with K=32 matmuls of
-sA x mr; [128,512] Sigmoid per parity (folded bias); sig*V on GPSIMD
into the fp8 ring; 3x3 conv in fp8 DoubleRow (12 matmuls, single
start=True on the first -- PSUM has_written clears are bank-wide); conv
bias folded into the residual: out = (q + cb) + cv on DVE; f32 store.
Consts and v load ride the Activation DMA queue; q loads + out stores on
the SP queue. V path mirrors the algebraic fold with an 8-chunk selector.
"""

import numpy as np
import ml_dtypes

import concourse.bass as bass
import concourse.bacc as bacc
import concourse.tile as tile
from concourse import mybir
from concourse.bass_utils import run_bass_kernel_spmd

F32 = mybir.dt.float32
BF16 = mybir.dt.bfloat16
FP8 = mybir.dt.float8e4
U32 = mybir.dt.uint32
I32 = mybir.dt.int32
AF = mybir.ActivationFunctionType
ALU = mybir.AluOpType
EPS = 1e-5
MAGIC = 0x5F3759DF
NPBF16 = ml_dtypes.bfloat16

_CACHE = {}


def _build_nc():
    nc = bacc.Bacc()
    q_d = nc.declare_dram_parameter("q", [128, 32768], BF16, isOutput=False)
    v_d = nc.declare_dram_parameter("v", [128, 4096], BF16, isOutput=False)
    A2_d = nc.declare_dram_parameter("A2", [128, 128], BF16, isOutput=False)
    cb_d = nc.declare_dram_parameter("cbias", [128, 1], F32, isOutput=False)
    cbbp_d = nc.declare_dram_parameter("cbbp", [128, 1], F32, isOutput=False)
    vwf_d = nc.declare_dram_parameter("vwf", [128, 128], BF16, isOutput=False)
    vbp_d = nc.declare_dram_parameter("vbp", [128, 1], F32, isOutput=False)
    cwt_d = nc.declare_dram_parameter("cwt8", [128, 1536], FP8, isOutput=False)
    sel_d = nc.declare_dram_parameter("sel", [128, 2048], BF16, isOutput=False)
    selv_d = nc.declare_dram_parameter("selv", [128, 64], BF16, isOutput=False)
    bcq_d = nc.declare_dram_parameter("bcq", [64, 2048], BF16, isOutput=False)
    nse_d = nc.declare_dram_parameter("nse", [64, 2048], BF16, isOutput=False)
    nso_d = nc.declare_dram_parameter("nso", [64, 2048], BF16, isOutput=False)
    bcv_d = nc.declare_dram_parameter("bcv", [8, 1024], BF16, isOutput=False)
    nsvm_d = nc.declare_dram_parameter("nsvm", [8, 1024], BF16, isOutput=False)
    out_d = nc.declare_dram_parameter("out", [128, 32768], F32, isOutput=True)

    with tile.TileContext(nc) as tc, \
         tc.tile_pool(name="const", bufs=1) as cpool, \
         tc.tile_pool(name="vwork", bufs=1) as vpool, \
         tc.tile_pool(name="qsq", bufs=4) as qsq_pool, \
         tc.tile_pool(name="bulk", bufs=1) as bk_pool, \
         tc.tile_pool(name="qs", bufs=4) as qs_pool, \
         tc.tile_pool(name="sig", bufs=4) as sig_pool, \
         tc.tile_pool(name="ring", bufs=1) as rg_pool, \
         tc.tile_pool(name="outp", bufs=3) as out_pool, \
         tc.tile_pool(name="ps_st", bufs=1, space="PSUM") as ps_st, \
         tc.tile_pool(name="ps_rr", bufs=2, space="PSUM") as ps_rr, \
         tc.tile_pool(name="ps_lg", bufs=2, space="PSUM") as ps_lg, \
         tc.tile_pool(name="ps_cv", bufs=2, space="PSUM") as ps_cv:

        def const_tile(shape, dtype, tag, src):
            t = cpool.tile(shape, dtype, tag=tag)
            nc.sync.dma_start(out=t, in_=src[:, :])
            return t

        A2_sb = const_tile([128, 128], BF16, "A2", A2_d)
        cb_sb = const_tile([128, 1], F32, "cb", cb_d)
        cbbp_sb = const_tile([128, 1], F32, "cbbp", cbbp_d)
        vwf_sb = const_tile([128, 128], BF16, "vwf", vwf_d)
        vbp_sb = const_tile([128, 1], F32, "vbp", vbp_d)
        cwt_sb = const_tile([128, 1536], FP8, "cwt", cwt_d)
        sel_sb = const_tile([128, 2048], BF16, "sel", sel_d)
        bcq_sb = const_tile([64, 2048], BF16, "bcq", bcq_d)
        nse_sb = const_tile([64, 2048], BF16, "nse", nse_d)
        nso_sb = const_tile([64, 2048], BF16, "nso", nso_d)
        bcv_sb = const_tile([8, 1024], BF16, "bcv", bcv_d)
        nsvm_sb = const_tile([8, 1024], BF16, "nsvm", nsvm_d)

        # all of q stays resident in SBUF as bf16 (64 KB of column space)
        qall = cpool.tile([128, 32768], BF16, tag="qall")
        # per-half LN stats: cols 0:512 = rr, 512:1024 = mu*rr;
        # partition 2j+par for local block j of the half
        rrmr0 = cpool.tile([32, 1024], BF16, tag="rrmr0")
        rrmr1 = cpool.tile([32, 1024], BF16, tag="rrmr1")
        rrmr2 = cpool.tile([32, 1024], BF16, tag="rrmr2")
        rrmr3 = cpool.tile([32, 1024], BF16, tag="rrmr3")
        rrmrQ = [rrmr0, rrmr1, rrmr2, rrmr3]
        rrv = cpool.tile([8, 1024], BF16, tag="rrv")
        V_sb = cpool.tile([128, 4096], F32, tag="V")
        # fp8 srow ring: 24 slots (row r -> slot r%24) + slot 24 duplicating
        # rows r%24==0 so tap pairs (23,24) stay contiguous for DoubleRow.
        ring = rg_pool.tile([128, 25 * 256], FP8, tag="ring")

        for k in range(2):
            (nc.sync if k % 2 == 0 else nc.scalar).dma_start(
                out=qall[:, k * 512:(k + 1) * 512],
                in_=q_d[:, k * 512:(k + 1) * 512])
        selv_sb = const_tile([128, 64], BF16, "selv", selv_d)
        for k in range(2, 10):
            (nc.sync if k % 2 == 0 else nc.scalar).dma_start(
                out=qall[:, k * 512:(k + 1) * 512],
                in_=q_d[:, k * 512:(k + 1) * 512])
        nc.sync.dma_start(out=sel_sb[:, 256:2048], in_=sel_d[:, 256:2048])
        vraw = vpool.tile([128, 4096], BF16, tag="vraw")
        nc.sync.dma_start(out=vraw, in_=v_d[:, :])
        for k in range(10, 32):
            (nc.sync if k % 2 == 0 else nc.scalar).dma_start(
                out=qall[:, k * 512:(k + 1) * 512],
                in_=q_d[:, k * 512:(k + 1) * 512])
        A2_sb = const_tile([128, 128], BF16, "A2", A2_d)
        cb_sb = const_tile([128, 1], F32, "cb", cb_d)
        cbbp_sb = const_tile([128, 1], F32, "cbbp", cbbp_d)
        vwf_sb = const_tile([128, 128], BF16, "vwf", vwf_d)
        vbp_sb = const_tile([128, 1], F32, "vbp", vbp_d)
        cwt_sb = const_tile([128, 1536], FP8, "cwt", cwt_d)
        bcq_sb = const_tile([64, 2048], BF16, "bcq", bcq_d)
        nse_sb = const_tile([64, 2048], BF16, "nse", nse_d)
        nso_sb = const_tile([64, 2048], BF16, "nso", nso_d)
        bcv_sb = const_tile([8, 1024], BF16, "bcv", bcv_d)
        nsvm_sb = const_tile([8, 1024], BF16, "nsvm", nsvm_d)
        for k in range(32, 64):
            (nc.sync if k % 2 == 0 else nc.scalar).dma_start(
                out=qall[:, k * 512:(k + 1) * 512],
                in_=q_d[:, k * 512:(k + 1) * 512])

        def stats_block(st, qtr, j16):
            mu_ps, sq_ps = st
            k = 16 * qtr + j16
            qk = qall[:, k * 512:(k + 1) * 512]
            qsq = qsq_pool.tile([128, 512], BF16, tag="qsq")
            nc.gpsimd.tensor_mul(qsq, qk, qk)
            selj = sel_sb[:, j16 * 64:j16 * 64 + 32]
            nc.tensor.matmul(mu_ps, selj, qk,
                             start=(j16 == 0), stop=(j16 == 15))
            nc.tensor.matmul(sq_ps, selj, qsq,
                             start=(j16 == 0), stop=(j16 == 15))

        def stats_quarter():
            mu_ps = ps_st.tile([32, 512], F32, tag="stm")
            sq_ps = ps_st.tile([32, 512], F32, tag="sts")
            return (mu_ps, sq_ps)

        def rsqrt_bulk(rr_out, mu_ps, sq_ps, np_, tag, muc=None):  # noqa
            """rr_out(bf16) = 1/sqrt(sq - mu^2 + eps) on [np_,512] tiles."""
            mu2 = bk_pool.tile([np_, 512], F32, tag=tag + "mu2")
            nc.scalar.activation(mu2, mu_ps, AF.Square)
            if muc is not None:
                nc.vector.tensor_copy(muc, mu_ps)
            vp = bk_pool.tile([np_, 512], F32, tag=tag + "vp")
            nc.vector.scalar_tensor_tensor(vp, sq_ps, EPS, mu2,
                                           ALU.add, ALU.subtract)
            y0 = bk_pool.tile([np_, 512], F32, tag=tag + "y0")
            nc.vector.tensor_scalar(y0.bitcast(U32), vp.bitcast(U32), 1, None,
                                    ALU.logical_shift_right)
            nc.vector.tensor_scalar(y0.bitcast(I32), y0.bitcast(I32),
                                    -1, None, ALU.bitwise_xor)
            nc.vector.tensor_scalar(y0.bitcast(I32), y0.bitcast(I32),
                                    MAGIC + 1, None, ALU.add)
            t_ = bk_pool.tile([np_, 512], F32, tag=tag + "t_")
            nc.gpsimd.tensor_mul(t_, y0, y0)
            nc.gpsimd.tensor_mul(t_, t_, vp)
            t2 = bk_pool.tile([np_, 512], F32, tag=tag + "t2")
            nc.vector.tensor_scalar(t2, t_, -0.5, 1.5, ALU.mult, ALU.add)
            nc.gpsimd.tensor_mul(rr_out, y0, t2)

        def bulk_quarter(st, rrmrX):
            mu_ps, sq_ps = st
            muc = bk_pool.tile([32, 512], F32, tag="qmuc")
            rsqrt_bulk(rrmrX[:, 0:512], mu_ps, sq_ps, 32, "q", muc=muc)
            nc.gpsimd.tensor_mul(rrmrX[:, 512:1024], muc,
                                 rrmrX[:, 0:512])

        def attn_pre(k):
            rrmrX = rrmrQ[k // 16]
            j2 = k % 16
            rr2 = ps_rr.tile([128, 512], F32, tag="rr")
            nc.tensor.matmul(rr2, bcq_sb[0:32, j2 * 128:(j2 + 1) * 128],
                             rrmrX[0:32, 0:512], start=True, stop=True)
            qs = qs_pool.tile([128, 512], BF16, tag="qs")
            nc.vector.tensor_mul(qs, qall[:, k * 512:(k + 1) * 512], rr2)
            return qs

        def attn_main(k, qs):
            rrmrX = rrmrQ[k // 16]
            j2 = k % 16
            mrs = rrmrX[0:32, 512:1024]
            lg_e = ps_lg.tile([128, 512], F32, tag="lg")
            nc.tensor.matmul(lg_e, A2_sb[0:64, :], qs[0:64, :],
                             start=True, stop=False)
            nc.tensor.matmul(lg_e, nse_sb[0:32, j2 * 128:(j2 + 1) * 128],
                             mrs, start=False, stop=True)
            lg_o = ps_lg.tile([128, 512], F32, tag="lg")
            nc.tensor.matmul(lg_o, A2_sb[64:128, :], qs[64:128, :],
                             start=True, stop=False)
            nc.tensor.matmul(lg_o, nso_sb[0:32, j2 * 128:(j2 + 1) * 128],
                             mrs, start=False, stop=True)
            sig_e = sig_pool.tile([128, 512], BF16, tag="sig")
            nc.scalar.activation(sig_e, lg_e, AF.Sigmoid, bias=cb_sb[:, 0:1])
            sig_o = sig_pool.tile([128, 512], BF16, tag="sig")
            nc.scalar.activation(sig_o, lg_o, AF.Sigmoid, bias=cb_sb[:, 0:1])
            vsl = V_sb[:, k * 64:(k + 1) * 64]
            vb_ap = vsl.rearrange("p c -> p c ()").broadcast_to([128, 64, 4])
            for r, (st_, half) in zip(
                    (4 * k, 4 * k + 1, 4 * k + 2, 4 * k + 3),
                    ((sig_e, 0), (sig_o, 0), (sig_e, 1), (sig_o, 1))):
                sig_ap = st_[:, half * 256:(half + 1) * 256].rearrange(
                    "p (c f) -> p c f", f=4)
                slots = [r % 24] + ([24] if r % 24 == 0 else [])
                for s in slots:
                    nc.gpsimd.tensor_mul(
                        ring[:, s * 256:(s + 1) * 256].rearrange(
                            "p (c f) -> p c f", f=4),
                        sig_ap, vb_ap)

        def conv_block(y0):
            cv = ps_cv.tile([128, 512], F32, tag="cv")
            for bi, dx in enumerate((1, 0, 2)):
                for tp in range(2):
                    pb = (bi * 2 + tp) * 256
                    last = (dx == 2 and tp == 1)
                    for p in range(2):
                        first = (bi == 0 and tp == 0 and p == 0)
                        rA = y0 + 2 * p - 1 + 2 * tp
                        base = p * 256
                        if rA < 0 or rA + 1 > 255:
                            # image edge: single valid tap, plain fp8 mm
                            kk = 1 if rA < 0 else 0
                            row = rA + kk
                            wt = cwt_sb[:, pb + kk * 128:pb + (kk + 1) * 128]
                            s = row % 24
                            rt = ring[:, s * 256:(s + 1) * 256]
                            if dx == 1:
                                nc.tensor.matmul(cv[:, base:base + 256],
                                                 wt, rt[:, 0:256],
                                                 start=first, stop=last)
                            elif dx == 0:
                                nc.tensor.matmul(cv[:, base + 1:base + 256],
                                                 wt, rt[:, 0:255],
                                                 start=False, stop=False)
                            else:
                                nc.tensor.matmul(cv[:, base:base + 255],
                                                 wt, rt[:, 1:256],
                                                 start=False, stop=last)
                            continue
                        sA_ = rA % 24
                        lhsT = cwt_sb[:, pb:pb + 256].rearrange(
                            "p (k m) -> p k m", k=2)
                        rhs2 = ring[:, sA_ * 256:sA_ * 256 + 512].rearrange(
                            "p (k n) -> p k n", k=2)
                        DR = mybir.MatmulPerfMode.DoubleRow
                        if dx == 1:
                            nc.tensor.matmul(cv[:, base:base + 256], lhsT,
                                             rhs2, start=first, stop=False,
                                             perf_mode=DR)
                        elif dx == 0:
                            nc.tensor.matmul(cv[:, base + 1:base + 256],
                                             lhsT, rhs2[:, :, 0:255],
                                             start=False, stop=False,
                                             perf_mode=DR)
                        else:
                            nc.tensor.matmul(cv[:, base:base + 255], lhsT,
                                             rhs2[:, :, 1:256], start=False,
                                             stop=last, perf_mode=DR)
            k = y0 // 4
            ot = out_pool.tile([128, 512], F32, tag="ot")
            # out = (q + conv_bias) + cv   (conv bias folded here)
            nc.vector.scalar_tensor_tensor(
                ot, qall[:, k * 512:(k + 1) * 512], cbbp_sb[:, 0:1], cv,
                ALU.add, ALU.add)
            nc.sync.dma_start(out=out_d[:, k * 512:(k + 1) * 512], in_=ot)

        # ---------------- quarter 0 stats ----------------
        st0 = stats_quarter()
        for j in range(16):
            stats_block(st0, 0, j)

        # ------- v stats (borrow ps_rr / ps_lg slots) ---
        muv_t = ps_rr.tile([128, 512], F32, tag="rr")
        sqv_t = ps_lg.tile([128, 512], F32, tag="lg")
        muv_ps = muv_t[0:8, :]
        sqv_ps = sqv_t[0:8, :]
        for c in range(8):
            sl = slice(c * 512, (c + 1) * 512)
            vsq = qsq_pool.tile([128, 512], BF16, tag="qsq")
            nc.gpsimd.tensor_mul(vsq, vraw[:, sl], vraw[:, sl])
            selc = selv_sb[:, c * 8:(c + 1) * 8]
            nc.tensor.matmul(muv_ps, selc, vraw[:, sl],
                             start=(c == 0), stop=(c == 7))
            nc.tensor.matmul(sqv_ps, selc, vsq,
                             start=(c == 0), stop=(c == 7))

        bulk_quarter(st0, rrmrQ[0])
        rsqrt_bulk(rrv[:, 0:512], muv_ps, sqv_ps, 8, "v")
        nc.vector.tensor_mul(rrv[:, 512:1024], muv_ps, rrv[:, 0:512])

        # ---------------- V path finish ----------------
        for c in range(8):
            sl = slice(c * 512, (c + 1) * 512)
            rrb = ps_rr.tile([128, 512], F32, tag="rr")
            nc.tensor.matmul(rrb, bcv_sb[:, c * 128:(c + 1) * 128],
                             rrv[0:8, 0:512], start=True, stop=True)
            vs = qs_pool.tile([128, 512], BF16, tag="qs")
            nc.vector.tensor_mul(vs, vraw[:, sl], rrb)
            vl = ps_lg.tile([128, 512], F32, tag="lg")
            nc.tensor.matmul(vl, vwf_sb, vs, start=True, stop=False)
            nc.tensor.matmul(vl, nsvm_sb[:, c * 128:(c + 1) * 128],
                             rrv[0:8, 512:1024], start=False, stop=True)
            nc.scalar.add(V_sb[:, sl], vl, vbp_sb[:, 0:1])

        # ------- main loop; next quarter's stats fill PE gaps -----------
        st_next = stats_quarter()
        qs_cur = attn_pre(0)
        for k in range(64):
            qs_nxt = attn_pre(k + 1) if k < 63 else None
            qtr = k // 16
            j2 = k % 16
            if qtr < 3 and j2 < 8:
                stats_block(st_next, qtr + 1, 2 * j2)
                stats_block(st_next, qtr + 1, 2 * j2 + 1)
            attn_main(k, qs_cur)
            qs_cur = qs_nxt
            if qtr < 3 and j2 == 8:
                bulk_quarter(st_next, rrmrQ[qtr + 1])
                if qtr < 2:
                    st_next = stats_quarter()
            if k >= 1:
                conv_block(4 * (k - 1))
        conv_block(4 * 63)

    nc.finalize()
    return nc


def _fold_weights(qW, qb, vW, vb, K, qn_g, qn_b, vn_g, vn_b, cW, cb):
    f = np.float32
    qW, qb, vW, vb, K = f(qW), f(qb), f(vW), f(vb), f(K)
    qn_g, qn_b, vn_g, vn_b, cW, cb = f(qn_g), f(qn_b), f(vn_g), f(vn_b), f(cW), f(cb)
    scale = np.float32(64.0 ** -0.5)
    qWf = qn_g[:, None] * qW.T                      # [c, co]
    bprime = qb + qW @ qn_b                         # [64]
    A = scale * (qWf @ K.T)                         # [64, 128]
    c_b = scale * (K @ bprime)                      # [128]  (sigmoid bias)
    sA = A.sum(axis=0)                              # [128]
    vWf = vn_g[:, None] * vW.T / 32.0               # [128, 128] (1/32 for fp8)
    vbp = (vb + vW @ vn_b) / 32.0                   # [128]
    svwf = vWf.sum(axis=0)                          # [128]
    cwt = np.zeros((128, 12, 128), np.float32)
    for bi, dx in enumerate((1, 0, 2)):
        for ti, t in enumerate((-1, 0, 1, 2)):
            blk = bi * 4 + ti
            if 0 <= t + 1 <= 2:
                cwt[:, blk, 0:64] = cW[:, :, t + 1, dx].T
            if 0 <= t <= 2:
                cwt[:, blk, 64:128] = cW[:, :, t, dx].T
    # fp8 DoubleRow layout: [128, 3dx, 2 tap-pairs, 2 k-tiles, 128], x32 to
    # sit in fp8e4m3's normal range (V is scaled by 1/32 to compensate).
    cwt8 = (cwt.reshape(128, 3, 2, 2, 128) * 32.0).astype(
        ml_dtypes.float8_e4m3)
    # stats selector: local block j -> rows 2j (even), 2j+1 (odd)
    sel = np.zeros((128, 32, 64), np.float32)
    for j in range(32):
        sel[0:64, j, 2 * j] = 1.0 / 64
        sel[64:128, j, 2 * j + 1] = 1.0 / 64
    selv = np.zeros((128, 8, 8), np.float32)
    for c in range(8):
        selv[:, c, c] = 1.0 / 128
    # phase-B row-selecting lhsTs, duplicated at partition offset 32 so
    # lhsT.base matches rhs.base for quadrant reads
    bcq = np.zeros((64, 16, 128), np.float32)
    nse = np.zeros((64, 16, 128), np.float32)
    nso = np.zeros((64, 16, 128), np.float32)
    for j2 in range(16):
        for off in (0, 32):
            bcq[off + 2 * j2, j2, 0:64] = 1.0
            bcq[off + 2 * j2 + 1, j2, 64:128] = 1.0
            nse[off + 2 * j2, j2, :] = -sA
            nso[off + 2 * j2 + 1, j2, :] = -sA
    bcv = np.zeros((8, 8, 128), np.float32)
    nsvm = np.zeros((8, 8, 128), np.float32)
    for c in range(8):
        bcv[c, c, :] = 1.0
        nsvm[c, c, :] = -svwf
    return {
        "A2": np.ascontiguousarray(
            np.concatenate([A, A], axis=0).astype(NPBF16)),
        "cbias": np.ascontiguousarray(c_b.reshape(128, 1)),
        "cbbp": np.ascontiguousarray(
            np.concatenate([cb, cb]).reshape(128, 1)),
        "vwf": np.ascontiguousarray(vWf.astype(NPBF16)),
        "vbp": np.ascontiguousarray(vbp.reshape(128, 1)),
        "cwt8": np.ascontiguousarray(cwt8.reshape(128, 1536)),
        "sel": np.ascontiguousarray(sel.reshape(128, 2048).astype(NPBF16)),
        "selv": np.ascontiguousarray(selv.reshape(128, 64).astype(NPBF16)),
        "bcq": np.ascontiguousarray(bcq.reshape(64, 2048).astype(NPBF16)),
        "nse": np.ascontiguousarray(nse.reshape(64, 2048).astype(NPBF16)),
        "nso": np.ascontiguousarray(nso.reshape(64, 2048).astype(NPBF16)),
        "bcv": np.ascontiguousarray(bcv.reshape(8, 1024).astype(NPBF16)),
        "nsvm": np.ascontiguousarray(nsvm.reshape(8, 1024).astype(NPBF16)),
    }


def _pack_q(qi):
    """[64,256,256] f32 -> [128,32768] bf16: partitions (ch, row-parity)."""
    qs = np.empty((128, 128, 256), np.float32)
    qs[0:64] = qi[:, 0::2, :]
    qs[64:128] = qi[:, 1::2, :]
    return np.ascontiguousarray(qs.reshape(128, 32768).astype(NPBF16))


def _unpack_out(r):
    """[128,32768] -> [64,256,256] undoing the row-parity packing."""
    arr = np.asarray(r, np.float32).reshape(128, 128, 256)
    out = np.empty((64, 256, 256), np.float32)
    out[:, 0::2, :] = arr[0:64]
    out[:, 1::2, :] = arr[64:128]
    return out


def _run(in_maps, trace=False, **kw):
    if "nc" not in _CACHE:
        _CACHE["nc"] = _build_nc()
    return run_bass_kernel_spmd(_CACHE["nc"], in_maps, list(range(8)),
                                trace=trace, **kw)


def kernel(q, v, qW, qb, vW, vb, K, qn_g, qn_b, vn_g, vn_b, cW, cb):
    base = _fold_weights(qW, qb, vW, vb, K, qn_g, qn_b, vn_g, vn_b, cW, cb)
    in_maps = []
    for i in range(8):
        m = dict(base)
        m["q"] = _pack_q(np.float32(q[i]))
        m["v"] = np.ascontiguousarray(
            np.float32(v[i]).reshape(128, 4096).astype(NPBF16))
        in_maps.append(m)
    res = _run(in_maps)
    outs = [_unpack_out(r["out"]) for r in res.results]
    return np.stack(outs)


# revision 27
# speedup vs baseline: 1.0082x; 1.0082x over previous
"""LocalPatchAttention Trainium2 kernel (v3: no-gather stats + half pipeline).

Data-parallel over batch B=8 across 8 NeuronCores (one image per core).
q and out live in DRAM as [128, 32768] with partitions = (channel,
row-parity): p<64 = channel p of even rows, p>=64 = odd rows; host packs
q to bf16 and unpacks f32 out.

LayerNorm is folded into the logits matmul:
  logits[v,px] = A^T(q*rr) - sA[v]*(mu[px]*rr[px]),  sA = column sums of A.

Stats are computed in two halves of 32 blocks (block = 2 row-pairs = 4
image rows = one [128,512] q tile). Per half, selector-lhsT matmuls
accumulate per-row mean into partitions 0:64 and E[q^2] into 64:128 of a
single shared PSUM bank; a ~10-op bulk DVE pipeline computes rr/mu*rr for
all 64 rows at once into rrmrX [64,1024] bf16 (cols 0:512 rr, 512:1024
mu*rr). Phase B reads each block's stat rows DIRECTLY from rrmrX with
K=32 row-selecting lhsT matmuls (rhs partition base 0 or 32; selector
tiles are duplicated at partition offset 32 to satisfy lhsT.base ==
rhs.base) -- no gather DMAs at all. Halves are pipelined: half 1's stats
matmuls fill PE gaps during half 0's phase B.

Per block B: one K=32 bcast matmul -> rr2 [128,512]; one DVE multiply
qs = q*rr2 (bf16); lg_{e,o} = A^T qs accumulated with K=32 matmuls of
-sA x mr; [128,512] Sigmoid per parity (folded bias); sig*V on GPSIMD
into the fp8 ring; 3x3 conv in fp8 DoubleRow (12 matmuls, single
start=True on the first -- PSUM has_written clears are bank-wide); conv
bias folded into the residual: out = (q + cb) + cv on DVE; f32 store.
Consts and v load ride the Activation DMA queue; q loads + out stores on
the SP queue. V path mirrors the algebraic fold with an 8-chunk selector.
"""

import numpy as np
import ml_dtypes

import concourse.bass as bass
import concourse.bacc as bacc
import concourse.tile as tile
from concourse import mybir
from concourse.bass_utils import run_bass_kernel_spmd

F32 = mybir.dt.float32
BF16 = mybir.dt.bfloat16
FP8 = mybir.dt.float8e4
U32 = mybir.dt.uint32
I32 = mybir.dt.int32
AF = mybir.ActivationFunctionType
ALU = mybir.AluOpType
EPS = 1e-5
MAGIC = 0x5F3759DF
NPBF16 = ml_dtypes.bfloat16

_CACHE = {}


def _build_nc():
    nc = bacc.Bacc()
    q_d = nc.declare_dram_parameter("q", [128, 32768], BF16, isOutput=False)
    v_d = nc.declare_dram_parameter("v", [128, 4096], BF16, isOutput=False)
    A2_d = nc.declare_dram_parameter("A2", [128, 128], BF16, isOutput=False)
    cb_d = nc.declare_dram_parameter("cbias", [128, 1], F32, isOutput=False)
    cbbp_d = nc.declare_dram_parameter("cbbp", [128, 1], F32, isOutput=False)
    vwf_d = nc.declare_dram_parameter("vwf", [128, 128], BF16, isOutput=False)
    vbp_d = nc.declare_dram_parameter("vbp", [128, 1], F32, isOutput=False)
    cwt_d = nc.declare_dram_parameter("cwt8", [128, 1536], FP8, isOutput=False)
    sel_d = nc.declare_dram_parameter("sel", [128, 2048], BF16, isOutput=False)
    selv_d = nc.declare_dram_parameter("selv", [128, 64], BF16, isOutput=False)
    bcq_d = nc.declare_dram_parameter("bcq", [64, 2048], BF16, isOutput=False)
    nse_d = nc.declare_dram_parameter("nse", [64, 2048], BF16, isOutput=False)
    nso_d = nc.declare_dram_parameter("nso", [64, 2048], BF16, isOutput=False)
    bcv_d = nc.declare_dram_parameter("bcv", [8, 1024], BF16, isOutput=False)
    nsvm_d = nc.declare_dram_parameter("nsvm", [8, 1024], BF16, isOutput=False)
    out_d = nc.declare_dram_parameter("out", [128, 32768], F32, isOutput=True)

    with tile.TileContext(nc) as tc, \
         tc.tile_pool(name="const", bufs=1) as cpool, \
         tc.tile_pool(name="vwork", bufs=1) as vpool, \
         tc.tile_pool(name="qsq", bufs=4) as qsq_pool, \
         tc.tile_pool(name="bulk", bufs=1) as bk_pool, \
         tc.tile_pool(name="qs", bufs=4) as qs_pool, \
         tc.tile_pool(name="sig", bufs=4) as sig_pool, \
         tc.tile_pool(name="ring", bufs=1) as rg_pool, \
         tc.tile_pool(name="outp", bufs=3) as out_pool, \
         tc.tile_pool(name="ps_st", bufs=1, space="PSUM") as ps_st, \
         tc.tile_pool(name="ps_rr", bufs=2, space="PSUM") as ps_rr, \
         tc.tile_pool(name="ps_lg", bufs=2, space="PSUM") as ps_lg, \
         tc.tile_pool(name="ps_cv", bufs=2, space="PSUM") as ps_cv:

        def const_tile(shape, dtype, tag, src):
            t = cpool.tile(shape, dtype, tag=tag)
            nc.sync.dma_start(out=t, in_=src[:, :])
            return t

        A2_sb = const_tile([128, 128], BF16, "A2", A2_d)
        cb_sb = const_tile([128, 1], F32, "cb", cb_d)
        cbbp_sb = const_tile([128, 1], F32, "cbbp", cbbp_d)
        vwf_sb = const_tile([128, 128], BF16, "vwf", vwf_d)
        vbp_sb = const_tile([128, 1], F32, "vbp", vbp_d)
        cwt_sb = const_tile([128, 1536], FP8, "cwt", cwt_d)
        sel_sb = const_tile([128, 2048], BF16, "sel", sel_d)
        bcq_sb = const_tile([64, 2048], BF16, "bcq", bcq_d)
        nse_sb = const_tile([64, 2048], BF16, "nse", nse_d)
        nso_sb = const_tile([64, 2048], BF16, "nso", nso_d)
        bcv_sb = const_tile([8, 1024], BF16, "bcv", bcv_d)
        nsvm_sb = const_tile([8, 1024], BF16, "nsvm", nsvm_d)

        # all of q stays resident in SBUF as bf16 (64 KB of column space)
        qall = cpool.tile([128, 32768], BF16, tag="qall")
        # per-half LN stats: cols 0:512 = rr, 512:1024 = mu*rr;
        # partition 2j+par for local block j of the half
        rrmr0 = cpool.tile([32, 1024], BF16, tag="rrmr0")
        rrmr1 = cpool.tile([32, 1024], BF16, tag="rrmr1")
        rrmr2 = cpool.tile([32, 1024], BF16, tag="rrmr2")
        rrmr3 = cpool.tile([32, 1024], BF16, tag="rrmr3")
        rrmrQ = [rrmr0, rrmr1, rrmr2, rrmr3]
        rrv = cpool.tile([8, 1024], BF16, tag="rrv")
        V_sb = cpool.tile([128, 4096], F32, tag="V")
        # fp8 srow ring: 24 slots (row r -> slot r%24) + slot 24 duplicating
        # rows r%24==0 so tap pairs (23,24) stay contiguous for DoubleRow.
        ring = rg_pool.tile([128, 25 * 256], FP8, tag="ring")

        for k in range(2):
            (nc.sync if k % 2 == 0 else nc.scalar).dma_start(
                out=qall[:, k * 512:(k + 1) * 512],
                in_=q_d[:, k * 512:(k + 1) * 512])
        selv_sb = const_tile([128, 64], BF16, "selv", selv_d)
        for k in range(2, 10):
            (nc.sync if k % 2 == 0 else nc.scalar).dma_start(
                out=qall[:, k * 512:(k + 1) * 512],
                in_=q_d[:, k * 512:(k + 1) * 512])
        vraw = vpool.tile([128, 4096], BF16, tag="vraw")
        nc.sync.dma_start(out=vraw, in_=v_d[:, :])
        for k in range(10, 32):
            (nc.sync if k % 2 == 0 else nc.scalar).dma_start(
                out=qall[:, k * 512:(k + 1) * 512],
                in_=q_d[:, k * 512:(k + 1) * 512])
        A2_sb = const_tile([128, 128], BF16, "A2", A2_d)
        cb_sb = const_tile([128, 1], F32, "cb", cb_d)
        cbbp_sb = const_tile([128, 1], F32, "cbbp", cbbp_d)
        vwf_sb = const_tile([128, 128], BF16, "vwf", vwf_d)
        vbp_sb = const_tile([128, 1], F32, "vbp", vbp_d)
        cwt_sb = const_tile([128, 1536], FP8, "cwt", cwt_d)
        bcq_sb = const_tile([64, 2048], BF16, "bcq", bcq_d)
        nse_sb = const_tile([64, 2048], BF16, "nse", nse_d)
        nso_sb = const_tile([64, 2048], BF16, "nso", nso_d)
        bcv_sb = const_tile([8, 1024], BF16, "bcv", bcv_d)
        nsvm_sb = const_tile([8, 1024], BF16, "nsvm", nsvm_d)
        for k in range(32, 64):
            (nc.sync if k % 2 == 0 else nc.scalar).dma_start(
                out=qall[:, k * 512:(k + 1) * 512],
                in_=q_d[:, k * 512:(k + 1) * 512])

        def stats_block(st, qtr, j16):
            mu_ps, sq_ps = st
            k = 16 * qtr + j16
            qk = qall[:, k * 512:(k + 1) * 512]
            qsq = qsq_pool.tile([128, 512], BF16, tag="qsq")
            nc.gpsimd.tensor_mul(qsq, qk, qk)
            selj = sel_sb[:, j16 * 64:j16 * 64 + 32]
            nc.tensor.matmul(mu_ps, selj, qk,
                             start=(j16 == 0), stop=(j16 == 15))
            nc.tensor.matmul(sq_ps, selj, qsq,
                             start=(j16 == 0), stop=(j16 == 15))

        def stats_quarter():
            mu_ps = ps_st.tile([32, 512], F32, tag="stm")
            sq_ps = ps_st.tile([32, 512], F32, tag="sts")
            return (mu_ps, sq_ps)

        def rsqrt_bulk(rr_out, mu_ps, sq_ps, np_, tag, muc=None):  # noqa
            """rr_out(bf16) = 1/sqrt(sq - mu^2 + eps) on [np_,512] tiles."""
            mu2 = bk_pool.tile([np_, 512], F32, tag=tag + "mu2")
            nc.scalar.activation(mu2, mu_ps, AF.Square)
            if muc is not None:
                nc.vector.tensor_copy(muc, mu_ps)
            vp = bk_pool.tile([np_, 512], F32, tag=tag + "vp")
            nc.vector.scalar_tensor_tensor(vp, sq_ps, EPS, mu2,
                                           ALU.add, ALU.subtract)
            y0 = bk_pool.tile([np_, 512], F32, tag=tag + "y0")
            nc.vector.tensor_scalar(y0.bitcast(U32), vp.bitcast(U32), 1, None,
                                    ALU.logical_shift_right)
            nc.vector.tensor_scalar(y0.bitcast(I32), y0.bitcast(I32),
                                    -1, None, ALU.bitwise_xor)
            nc.vector.tensor_scalar(y0.bitcast(I32), y0.bitcast(I32),
                                    MAGIC + 1, None, ALU.add)
            t_ = bk_pool.tile([np_, 512], F32, tag=tag + "t_")
            nc.gpsimd.tensor_mul(t_, y0, y0)
            nc.gpsimd.tensor_mul(t_, t_, vp)
            t2 = bk_pool.tile([np_, 512], F32, tag=tag + "t2")
            nc.vector.tensor_scalar(t2, t_, -0.5, 1.5, ALU.mult, ALU.add)
            nc.gpsimd.tensor_mul(rr_out, y0, t2)

        def bulk_quarter(st, rrmrX):
            mu_ps, sq_ps = st
            muc = bk_pool.tile([32, 512], F32, tag="qmuc")
            rsqrt_bulk(rrmrX[:, 0:512], mu_ps, sq_ps, 32, "q", muc=muc)
            nc.gpsimd.tensor_mul(rrmrX[:, 512:1024], muc,
                                 rrmrX[:, 0:512])

        def attn_pre(k):
            rrmrX = rrmrQ[k // 16]
            j2 = k % 16
            rr2 = ps_rr.tile([128, 512], F32, tag="rr")
            nc.tensor.matmul(rr2, bcq_sb[0:32, j2 * 128:(j2 + 1) * 128],
                             rrmrX[0:32, 0:512], start=True, stop=True)
            qs = qs_pool.tile([128, 512], BF16, tag="qs")
            nc.vector.tensor_mul(qs, qall[:, k * 512:(k + 1) * 512], rr2)
            return qs

        def attn_main(k, qs):
            rrmrX = rrmrQ[k // 16]
            j2 = k % 16
            mrs = rrmrX[0:32, 512:1024]
            lg_e = ps_lg.tile([128, 512], F32, tag="lg")
            nc.tensor.matmul(lg_e, A2_sb[0:64, :], qs[0:64, :],
                             start=True, stop=False)
            nc.tensor.matmul(lg_e, nse_sb[0:32, j2 * 128:(j2 + 1) * 128],
                             mrs, start=False, stop=True)
            lg_o = ps_lg.tile([128, 512], F32, tag="lg")
            nc.tensor.matmul(lg_o, A2_sb[64:128, :], qs[64:128, :],
                             start=True, stop=False)
            nc.tensor.matmul(lg_o, nso_sb[0:32, j2 * 128:(j2 + 1) * 128],
                             mrs, start=False, stop=True)
            sig_e = sig_pool.tile([128, 512], BF16, tag="sig")
            nc.scalar.activation(sig_e, lg_e, AF.Sigmoid, bias=cb_sb[:, 0:1])
            sig_o = sig_pool.tile([128, 512], BF16, tag="sig")
            nc.scalar.activation(sig_o, lg_o, AF.Sigmoid, bias=cb_sb[:, 0:1])
            vsl = V_sb[:, k * 64:(k + 1) * 64]
            vb_ap = vsl.rearrange("p c -> p c ()").broadcast_to([128, 64, 4])
            for r, (st_, half) in zip(
                    (4 * k, 4 * k + 1, 4 * k + 2, 4 * k + 3),
                    ((sig_e, 0), (sig_o, 0), (sig_e, 1), (sig_o, 1))):
                sig_ap = st_[:, half * 256:(half + 1) * 256].rearrange(
                    "p (c f) -> p c f", f=4)
                slots = [r % 24] + ([24] if r % 24 == 0 else [])
                for s in slots:
                    nc.gpsimd.tensor_mul(
                        ring[:, s * 256:(s + 1) * 256].rearrange(
                            "p (c f) -> p c f", f=4),
                        sig_ap, vb_ap)

        def conv_block(y0):
            cv = ps_cv.tile([128, 512], F32, tag="cv")
            for bi, dx in enumerate((1, 0, 2)):
                for tp in range(2):
                    pb = (bi * 2 + tp) * 256
                    last = (dx == 2 and tp == 1)
                    for p in range(2):
                        first = (bi == 0 and tp == 0 and p == 0)
                        rA = y0 + 2 * p - 1 + 2 * tp
                        base = p * 256
                        if rA < 0 or rA + 1 > 255:
                            # image edge: single valid tap, plain fp8 mm
                            kk = 1 if rA < 0 else 0
                            row = rA + kk
                            wt = cwt_sb[:, pb + kk * 128:pb + (kk + 1) * 128]
                            s = row % 24
                            rt = ring[:, s * 256:(s + 1) * 256]
                            if dx == 1:
                                nc.tensor.matmul(cv[:, base:base + 256],
                                                 wt, rt[:, 0:256],
                                                 start=first, stop=last)
                            elif dx == 0:
                                nc.tensor.matmul(cv[:, base + 1:base + 256],
                                                 wt, rt[:, 0:255],
                                                 start=False, stop=False)
                            else:
                                nc.tensor.matmul(cv[:, base:base + 255],
                                                 wt, rt[:, 1:256],
                                                 start=False, stop=last)
                            continue
                        sA_ = rA % 24
                        lhsT = cwt_sb[:, pb:pb + 256].rearrange(
                            "p (k m) -> p k m", k=2)
                        rhs2 = ring[:, sA_ * 256:sA_ * 256 + 512].rearrange(
                            "p (k n) -> p k n", k=2)
                        DR = mybir.MatmulPerfMode.DoubleRow
                        if dx == 1:
                            nc.tensor.matmul(cv[:, base:base + 256], lhsT,
                                             rhs2, start=first, stop=False,
                                             perf_mode=DR)
                        elif dx == 0:
                            nc.tensor.matmul(cv[:, base + 1:base + 256],
                                             lhsT, rhs2[:, :, 0:255],
                                             start=False, stop=False,
                                             perf_mode=DR)
                        else:
                            nc.tensor.matmul(cv[:, base:base + 255], lhsT,
                                             rhs2[:, :, 1:256], start=False,
                                             stop=last, perf_mode=DR)
            k = y0 // 4
            ot = out_pool.tile([128, 512], F32, tag="ot")
            # out = (q + conv_bias) + cv   (conv bias folded here)
            nc.vector.scalar_tensor_tensor(
                ot, qall[:, k * 512:(k + 1) * 512], cbbp_sb[:, 0:1], cv,
                ALU.add, ALU.add)
            nc.sync.dma_start(out=out_d[:, k * 512:(k + 1) * 512], in_=ot)

        # ---------------- quarter 0 stats ----------------
        st0 = stats_quarter()
        for j in range(16):
            stats_block(st0, 0, j)

        # ------- v stats (borrow ps_rr / ps_lg slots) ---
        muv_t = ps_rr.tile([128, 512], F32, tag="rr")
        sqv_t = ps_lg.tile([128, 512], F32, tag="lg")
        muv_ps = muv_t[0:8, :]
        sqv_ps = sqv_t[0:8, :]
        for c in range(8):
            sl = slice(c * 512, (c + 1) * 512)
            vsq = qsq_pool.tile([128, 512], BF16, tag="qsq")
            nc.gpsimd.tensor_mul(vsq, vraw[:, sl], vraw[:, sl])
            selc = selv_sb[:, c * 8:(c + 1) * 8]
            nc.tensor.matmul(muv_ps, selc, vraw[:, sl],
                             start=(c == 0), stop=(c == 7))
            nc.tensor.matmul(sqv_ps, selc, vsq,
                             start=(c == 0), stop=(c == 7))

        bulk_quarter(st0, rrmrQ[0])
        rsqrt_bulk(rrv[:, 0:512], muv_ps, sqv_ps, 8, "v")
        nc.vector.tensor_mul(rrv[:, 512:1024], muv_ps, rrv[:, 0:512])

        # ---------------- V path finish ----------------
        for c in range(8):
            sl = slice(c * 512, (c + 1) * 512)
            rrb = ps_rr.tile([128, 512], F32, tag="rr")
            nc.tensor.matmul(rrb, bcv_sb[:, c * 128:(c + 1) * 128],
                             rrv[0:8, 0:512], start=True, stop=True)
            vs = qs_pool.tile([128, 512], BF16, tag="qs")
            nc.vector.tensor_mul(vs, vraw[:, sl], rrb)
            vl = ps_lg.tile([128, 512], F32, tag="lg")
            nc.tensor.matmul(vl, vwf_sb, vs, start=True, stop=False)
            nc.tensor.matmul(vl, nsvm_sb[:, c * 128:(c + 1) * 128],
                             rrv[0:8, 512:1024], start=False, stop=True)
            nc.scalar.add(V_sb[:, sl], vl, vbp_sb[:, 0:1])

        # ------- main loop; next quarter's stats fill PE gaps -----------
        st_next = stats_quarter()
        qs_cur = attn_pre(0)
        for k in range(64):
            qs_nxt = attn_pre(k + 1) if k < 63 else None
            qtr = k // 16
            j2 = k % 16
            if qtr < 3 and j2 < 8:
                stats_block(st_next, qtr + 1, 2 * j2)
                stats_block(st_next, qtr + 1, 2 * j2 + 1)
            attn_main(k, qs_cur)
            qs_cur = qs_nxt
            if qtr < 3 and j2 == 8:
                bulk_quarter(st_next, rrmrQ[qtr + 1])
                if qtr < 2:
                    st_next = stats_quarter()
            if k >= 1:
                conv_block(4 * (k - 1))
        conv_block(4 * 63)

    nc.finalize()
    return nc


def _fold_weights(qW, qb, vW, vb, K, qn_g, qn_b, vn_g, vn_b, cW, cb):
    f = np.float32
    qW, qb, vW, vb, K = f(qW), f(qb), f(vW), f(vb), f(K)
    qn_g, qn_b, vn_g, vn_b, cW, cb = f(qn_g), f(qn_b), f(vn_g), f(vn_b), f(cW), f(cb)
    scale = np.float32(64.0 ** -0.5)
    qWf = qn_g[:, None] * qW.T                      # [c, co]
    bprime = qb + qW @ qn_b                         # [64]
    A = scale * (qWf @ K.T)                         # [64, 128]
    c_b = scale * (K @ bprime)                      # [128]  (sigmoid bias)
    sA = A.sum(axis=0)                              # [128]
    vWf = vn_g[:, None] * vW.T / 32.0               # [128, 128] (1/32 for fp8)
    vbp = (vb + vW @ vn_b) / 32.0                   # [128]
    svwf = vWf.sum(axis=0)                          # [128]
    cwt = np.zeros((128, 12, 128), np.float32)
    for bi, dx in enumerate((1, 0, 2)):
        for ti, t in enumerate((-1, 0, 1, 2)):
            blk = bi * 4 + ti
            if 0 <= t + 1 <= 2:
                cwt[:, blk, 0:64] = cW[:, :, t + 1, dx].T
            if 0 <= t <= 2:
                cwt[:, blk, 64:128] = cW[:, :, t, dx].T
    # fp8 DoubleRow layout: [128, 3dx, 2 tap-pairs, 2 k-tiles, 128], x32 to
    # sit in fp8e4m3's normal range (V is scaled by 1/32 to compensate).
    cwt8 = (cwt.reshape(128, 3, 2, 2, 128) * 32.0).astype(
        ml_dtypes.float8_e4m3)
    # stats selector: local block j -> rows 2j (even), 2j+1 (odd)
    sel = np.zeros((128, 32, 64), np.float32)
    for j in range(32):
        sel[0:64, j, 2 * j] = 1.0 / 64
        sel[64:128, j, 2 * j + 1] = 1.0 / 64
    selv = np.zeros((128, 8, 8), np.float32)
    for c in range(8):
        selv[:, c, c] = 1.0 / 128
    # phase-B row-selecting lhsTs, duplicated at partition offset 32 so
    # lhsT.base matches rhs.base for quadrant reads
    bcq = np.zeros((64, 16, 128), np.float32)
    nse = np.zeros((64, 16, 128), np.float32)
    nso = np.zeros((64, 16, 128), np.float32)
    for j2 in range(16):
        for off in (0, 32):
            bcq[off + 2 * j2, j2, 0:64] = 1.0
            bcq[off + 2 * j2 + 1, j2, 64:128] = 1.0
            nse[off + 2 * j2, j2, :] = -sA
            nso[off + 2 * j2 + 1, j2, :] = -sA
    bcv = np.zeros((8, 8, 128), np.float32)
    nsvm = np.zeros((8, 8, 128), np.float32)
    for c in range(8):
        bcv[c, c, :] = 1.0
        nsvm[c, c, :] = -svwf
    return {
        "A2": np.ascontiguousarray(
            np.concatenate([A, A], axis=0).astype(NPBF16)),
        "cbias": np.ascontiguousarray(c_b.reshape(128, 1)),
        "cbbp": np.ascontiguousarray(
            np.concatenate([cb, cb]).reshape(128, 1)),
        "vwf": np.ascontiguousarray(vWf.astype(NPBF16)),
        "vbp": np.ascontiguousarray(vbp.reshape(128, 1)),
        "cwt8": np.ascontiguousarray(cwt8.reshape(128, 1536)),
        "sel": np.ascontiguousarray(sel.reshape(128, 2048).astype(NPBF16)),
        "selv": np.ascontiguousarray(selv.reshape(128, 64).astype(NPBF16)),
        "bcq": np.ascontiguousarray(bcq.reshape(64, 2048).astype(NPBF16)),
        "nse": np.ascontiguousarray(nse.reshape(64, 2048).astype(NPBF16)),
        "nso": np.ascontiguousarray(nso.reshape(64, 2048).astype(NPBF16)),
        "bcv": np.ascontiguousarray(bcv.reshape(8, 1024).astype(NPBF16)),
        "nsvm": np.ascontiguousarray(nsvm.reshape(8, 1024).astype(NPBF16)),
    }


def _pack_q(qi):
    """[64,256,256] f32 -> [128,32768] bf16: partitions (ch, row-parity)."""
    qs = np.empty((128, 128, 256), np.float32)
    qs[0:64] = qi[:, 0::2, :]
    qs[64:128] = qi[:, 1::2, :]
    return np.ascontiguousarray(qs.reshape(128, 32768).astype(NPBF16))


def _unpack_out(r):
    """[128,32768] -> [64,256,256] undoing the row-parity packing."""
    arr = np.asarray(r, np.float32).reshape(128, 128, 256)
    out = np.empty((64, 256, 256), np.float32)
    out[:, 0::2, :] = arr[0:64]
    out[:, 1::2, :] = arr[64:128]
    return out


def _run(in_maps, trace=False, **kw):
    if "nc" not in _CACHE:
        _CACHE["nc"] = _build_nc()
    return run_bass_kernel_spmd(_CACHE["nc"], in_maps, list(range(8)),
                                trace=trace, **kw)


def kernel(q, v, qW, qb, vW, vb, K, qn_g, qn_b, vn_g, vn_b, cW, cb):
    base = _fold_weights(qW, qb, vW, vb, K, qn_g, qn_b, vn_g, vn_b, cW, cb)
    in_maps = []
    for i in range(8):
        m = dict(base)
        m["q"] = _pack_q(np.float32(q[i]))
        m["v"] = np.ascontiguousarray(
            np.float32(v[i]).reshape(128, 4096).astype(NPBF16))
        in_maps.append(m)
    res = _run(in_maps)
    outs = [_unpack_out(r["out"]) for r in res.results]
    return np.stack(outs)
